# revision 6
# baseline (speedup 1.0000x reference)
"""DeepseekV2 MLA attention prefill on 8 trn2 NeuronCores (Bass/Tile).

Sharding: tensor-parallel over heads (2 heads/core) with sequence-split
low-rank a-projections.

  Stage A (per core, its 256-row sequence slice, all in transposed layout —
    features on partitions — so no on-device transposes are needed):
    q_lora.T, kv_lora.T, k_pe.T (+RoPE).  RMSNorm via ones-matmul partition
    reductions.
  AllGather kv latents, then q latents (kv first so stage-B kv work overlaps
    the q projection pass).
  Stage B: kv_b/q_b projections of the gathered latents, q RoPE
    (de-interleaved pair layout, 32-row block swap via permutation matmul),
    causal attention with transposed scores: exp on ACT with the softmax scale
    folded in, column sums via ones-matmul, multiplicative causal masks on
    diagonal tiles, PV accumulation, normalization via partition-broadcast
    reciprocal sums.
  AllGather attention outputs per q-tile, column-sharded o_proj, host concat.

All matmuls in float32r (single-pass PE mode, ~1.5e-4 rel err).
"""
import sys

sys.path.insert(0, "/opt/trn_rl_repo")

import math

import numpy as np

import concourse.bass as bass
import concourse.mybir as mybir
import concourse.tile as tile
from concourse import bacc
from concourse.bass_utils import run_bass_kernel_spmd

F32 = mybir.dt.float32
F32R = mybir.dt.float32r

S = 2048
HIDDEN = 5120
H = 16
NOPE, ROPE, V = 128, 64, 128
QK = NOPE + ROPE
QLORA, KVLORA = 1536, 512
EPS = 1e-6
BASE, FACTOR = 10000.0, 40.0
BETA_FAST, BETA_SLOW = 32.0, 1.0
ORIG_MAX = 4096
MSCALE = MSCALE_ALL = 0.707

NC = 8
HL = H // NC          # 2 local heads
RC = S // NC          # 256 rows per core
OC = HIDDEN // NC     # 640 output cols per core
QT = 512
NQT = S // QT         # 4
KCH = S // 128        # 16
QFC = QLORA // 128    # 12
KVFC = KVLORA // 128  # 4
HKC = HIDDEN // 128   # 40


def _yarn_get_mscale(scale, mscale):
    if scale <= 1:
        return 1.0
    return 0.1 * mscale * math.log(scale) + 1.0


SCALING = QK ** -0.5 * _yarn_get_mscale(FACTOR, MSCALE_ALL) ** 2
COS_SIN_MSCALE = _yarn_get_mscale(FACTOR, MSCALE) / _yarn_get_mscale(FACTOR, MSCALE_ALL)


def _yarn_inv_freq():
    half = ROPE // 2
    i = np.arange(half, dtype=np.float64)
    pos_freqs = BASE ** (2.0 * i / ROPE)
    inv_extra = 1.0 / pos_freqs
    inv_interp = 1.0 / (FACTOR * pos_freqs)

    def corr_dim(n_rot):
        return ROPE * math.log(ORIG_MAX / (n_rot * 2 * math.pi)) / (2 * math.log(BASE))

    low = max(math.floor(corr_dim(BETA_FAST)), 0)
    high = min(math.ceil(corr_dim(BETA_SLOW)), ROPE - 1)
    ramp = np.clip((i - low) / max(high - low, 1e-3), 0.0, 1.0)
    inv_freq_mask = 1.0 - ramp
    return (inv_interp * (1.0 - inv_freq_mask) + inv_extra * inv_freq_mask).astype(
        np.float32
    )


Exp = mybir.ActivationFunctionType.Exp
Sqrt = mybir.ActivationFunctionType.Sqrt
Square = mybir.ActivationFunctionType.Square
MUL = mybir.AluOpType.mult


def _allgather(nc, tc, src, dst):
    with tc.tile_critical():
        with nc.semaphore() as sem:
            nc.gpsimd.collective_compute(
                "AllGather", mybir.AluOpType.bypass,
                ins=[src.ap()], outs=[dst.ap()],
                replica_groups=[list(range(NC))],
            ).then_inc(sem, 1)
            nc.gpsimd.wait_ge(sem, 1)


def _build_module():
    nc = bacc.Bacc(None)

    def param(name, shape, dtype=F32R):
        return nc.declare_dram_parameter(name, list(shape), dtype, isOutput=False)

    P = {}
    P["hidden_t"] = param("hidden_t", [HIDDEN, RC])
    P["w_q_a"] = param("w_q_a", [HIDDEN, QLORA])
    P["w_kv_a"] = param("w_kv_a", [HIDDEN, KVLORA + 128])
    P["lnw_q"] = param("lnw_q", [128, QFC], F32)
    P["lnw_kv"] = param("lnw_kv", [128, KVFC], F32)
    P["w_q_b"] = param("w_q_b", [QLORA, 3 * 128])
    P["w_kv_b"] = param("w_kv_b", [KVLORA, 4 * 128])
    P["w_o"] = param("w_o", [H * V, OC])
    P["a64"] = param("a64", [64, RC], F32)
    P["b64"] = param("b64", [64, RC], F32)
    P["a128"] = param("a128", [128, S], F32)
    P["b128"] = param("b128", [128, S], F32)
    P["perm64"] = param("perm64", [128, 64])
    P["perm128"] = param("perm128", [128, 128])
    P["ones_p"] = param("ones_p", [128, 1])
    P["maskt"] = param("maskt", [128, 4, QT], F32)
    y = nc.declare_dram_parameter("y", [S, OC], F32, isOutput=True)

    D = {}
    D["agq_in"] = nc.dram_tensor("agq_in", [QLORA, RC], F32)
    D["agkv_in"] = nc.dram_tensor("agkv_in", [KVLORA + 64, RC], F32)
    D["agq_out"] = nc.dram_tensor("agq_out", [NC * QLORA, RC], F32,
                                  addr_space="Shared")
    D["agkv_out"] = nc.dram_tensor("agkv_out", [NC * (KVLORA + 64), RC], F32,
                                   addr_space="Shared")
    D["ag2_in"] = [nc.dram_tensor(f"ag2_in{t}", [HL * V, QT], F32)
                   for t in range(NQT)]
    D["ag2_out"] = [nc.dram_tensor(f"ag2_out{t}", [H * V, QT], F32,
                                   addr_space="Shared") for t in range(NQT)]

    with tile.TileContext(nc) as tc:
        _body(nc, tc, P, D, y)
    nc.compile()
    return nc


def _body(nc, tc, P, D, y):
    wqa_r = P["w_q_a"].ap().rearrange("(kc p) f -> kc p f", p=128)
    wkva_r = P["w_kv_a"].ap().rearrange("(kc p) f -> kc p f", p=128)
    agq_w = D["agq_in"].ap().rearrange("(fc p) r -> fc p r", p=128)
    agkv_w = D["agkv_in"].ap()[0:KVLORA, :].rearrange("(x p) r -> x p r", p=128)
    agq_r = D["agq_out"].ap().rearrange("(r fc p) c -> r fc p c", r=NC, p=128)
    agkv_r = D["agkv_out"].ap().rearrange("(r x) c -> r x c", r=NC)

    with tc.tile_pool(name="singles", bufs=1) as singles, \
         tc.tile_pool(name="smalls", bufs=2) as smalls, \
         tc.tile_pool(name="rows_p", bufs=2) as rows_p:
        # ----- constants -----
        ones_sb = singles.tile([128, 1], F32R)
        nc.sync.dma_start(out=ones_sb, in_=P["ones_p"].ap())
        perm64_sb = singles.tile([128, 64], F32R)
        nc.sync.dma_start(out=perm64_sb, in_=P["perm64"].ap())
        perm128_sb = singles.tile([128, 128], F32R)
        nc.sync.dma_start(out=perm128_sb, in_=P["perm128"].ap())
        mask_sb = singles.tile([128, 4, QT], F32)
        nc.sync.dma_start(out=mask_sb, in_=P["maskt"].ap())
        lnwq_sb = singles.tile([128, QFC], F32)
        nc.sync.dma_start(out=lnwq_sb, in_=P["lnw_q"].ap())
        lnwkv_sb = singles.tile([128, KVFC], F32)
        nc.sync.dma_start(out=lnwkv_sb, in_=P["lnw_kv"].ap())
        eps_sb = singles.tile([1, 1], F32)
        nc.vector.memset(eps_sb, EPS)

        # ----- long-lived stage-B tensors -----
        qfT = singles.tile([128, HL, S], F32R)
        knT = singles.tile([128, HL, S], F32R)
        v_sb = singles.tile([128, KCH, HL * V], F32R)
        kpe_lo = singles.tile([128, S], F32R)
        kpe_hi = singles.tile([128, S], F32R)
        qropeP = singles.tile([128, S], F32R)

        def make_rstd(ss_ps, dim, tag):
            rstd0 = rows_p.tile([1, RC], F32, tag="rstd0")
            nc.scalar.activation(out=rstd0, in_=ss_ps, func=Sqrt,
                                 scale=1.0 / dim, bias=eps_sb)
            rstd = rows_p.tile([1, RC], F32, tag="rstd")
            nc.vector.reciprocal(out=rstd, in_=rstd0)
            rstd_b = smalls.tile([128, RC], F32, tag="rstd_b")
            nc.gpsimd.partition_broadcast(rstd_b, rstd)
            return rstd_b

        # ================= STAGE A (+ overlapped B-kv) =================
        with tc.tile_pool(name="sb_a", bufs=1) as sb_a, \
             tc.tile_pool(name="wstr", bufs=3) as wstr, \
             tc.tile_pool(name="anorm", bufs=2) as anorm:
            hid_sb = sb_a.tile([128, HKC, RC], F32R)
            nc.sync.dma_start(
                out=hid_sb,
                in_=P["hidden_t"].ap().rearrange("(kc p) r -> p kc r", p=128),
            )
            a64_sb = sb_a.tile([64, RC], F32)
            nc.sync.dma_start(out=a64_sb, in_=P["a64"].ap())
            b64_sb = sb_a.tile([64, RC], F32)
            nc.sync.dma_start(out=b64_sb, in_=P["b64"].ap())

            def a_group(ps_pool, w_r, fc0, ss_ps, first_ss, last_ss, ptag):
                # two separate psum tiles: interleaved accumulation chains must
                # not share a PSUM bank (start= clears the whole bank)
                psA = []
                for j in range(2):
                    psA_j = ps_pool.tile([128, RC], F32, tag=f"{ptag}{j}")
                    psA.append(psA_j)
                for kc in range(HKC):
                    w_sb = wstr.tile([128, 2 * 128], F32R, tag="wA")
                    nc.sync.dma_start(
                        out=w_sb, in_=w_r[kc][:, fc0 * 128:(fc0 + 2) * 128]
                    )
                    for j in range(2):
                        nc.tensor.matmul(
                            psA[j], w_sb[:, j * 128:(j + 1) * 128],
                            hid_sb[:, kc, :],
                            start=(kc == 0), stop=(kc == HKC - 1),
                        )
                for j in range(2):
                    sqf = smalls.tile([128, RC], F32R, tag="sqf")
                    nc.scalar.activation(out=sqf, in_=psA[j], func=Square)
                    nc.tensor.matmul(
                        ss_ps, ones_sb, sqf,
                        start=(fc0 + j == first_ss), stop=(fc0 + j == last_ss),
                        skip_group_check=True,
                    )
                return psA

            # ---------- kv a-pass ----------
            with tc.tile_pool(name="ps_akv", bufs=2, space="PSUM") as ps_akv, \
                 tc.tile_pool(name="ps_akv1", bufs=1, space="PSUM") as ps_akv1:
                ss_kv = ps_akv1.tile([1, RC], F32, tag="ss_kv")
                kv_ps = []
                for g in range(KVFC // 2):
                    kv_psA = a_group(ps_akv, wkva_r, g * 2, ss_kv, 0, KVFC - 1,
                                     "psA_kv")
                    kv_ps.append(kv_psA)
                ps_kpe = ps_akv1.tile([128, RC], F32, tag="ps_kpe")
                for kc in range(HKC):
                    w_sb = wstr.tile([128, 128], F32R, tag="wA64")
                    nc.sync.dma_start(
                        out=w_sb, in_=wkva_r[kc][:, KVLORA:KVLORA + 128]
                    )
                    nc.tensor.matmul(ps_kpe, w_sb, hid_sb[:, kc, :],
                                     start=(kc == 0), stop=(kc == HKC - 1))
                rstd_kv = make_rstd(ss_kv, KVLORA, "kv")
                for g in range(KVFC // 2):
                    for j in range(2):
                        nrm = anorm.tile([128, RC], F32, tag="nrm")
                        nc.vector.scalar_tensor_tensor(
                            out=nrm, in0=kv_ps[g][j],
                            scalar=lnwkv_sb[:, g * 2 + j:g * 2 + j + 1],
                            in1=rstd_kv, op0=MUL, op1=MUL,
                        )
                        nc.sync.dma_start(out=agkv_w[g * 2 + j], in_=nrm)
                # k_pe rope
                kpe_raw = smalls.tile([128, RC], F32R, tag="kpe_raw")
                nc.vector.tensor_copy(out=kpe_raw, in_=ps_kpe)
                ps_sw = ps_akv1.tile([64, RC], F32, tag="kpesw")
                nc.tensor.matmul(ps_sw, perm64_sb, kpe_raw, start=True, stop=True)
                t1 = smalls.tile([64, RC], F32, tag="kpet1")
                nc.vector.tensor_mul(t1, kpe_raw[0:64, :].bitcast(F32), a64_sb)
                t2 = smalls.tile([64, RC], F32, tag="kpet2")
                nc.vector.tensor_mul(t2, ps_sw, b64_sb)
                kroped = smalls.tile([64, RC], F32, tag="kroped")
                nc.vector.tensor_add(kroped, t1, t2)
                nc.sync.dma_start(
                    out=D["agkv_in"].ap()[KVLORA:KVLORA + 64, :], in_=kroped
                )

            _allgather(nc, tc, D["agkv_in"], D["agkv_out"])

            # ---------- B-kv (overlaps q a-pass) ----------
            with tc.tile_pool(name="wkvb_p", bufs=1) as wkvb_p, \
                 tc.tile_pool(name="blk_kv", bufs=2) as blk_kv, \
                 tc.tile_pool(name="ps_bkv", bufs=1, space="PSUM") as ps_bkv:
                wkvb_sb = wkvb_p.tile([128, KVFC, 4 * 128], F32R)
                nc.sync.dma_start(
                    out=wkvb_sb,
                    in_=P["w_kv_b"].ap().rearrange("(kc p) m -> p kc m", p=128),
                )
                zf = smalls.tile([128, QT], F32, tag="zf")
                nc.vector.memset(zf, 0.0)
                for t in range(NQT):
                    qs = slice(t * QT, (t + 1) * QT)
                    nc.vector.tensor_copy(out=kpe_lo[:, qs], in_=zf)
                    nc.vector.tensor_copy(out=kpe_hi[:, qs], in_=zf)
                for r in range(NC):
                    cs = slice(r * RC, (r + 1) * RC)
                    kvn_r = blk_kv.tile([128, KVFC, RC], F32R, tag="kvn")
                    nc.gpsimd.dma_start(
                        out=kvn_r,
                        in_=agkv_r[r, 0:KVLORA, :].rearrange(
                            "(fc p) c -> p fc c", p=128
                        ),
                    )
                    nc.gpsimd.dma_start(
                        out=kpe_lo[0:64, cs],
                        in_=agkv_r[r, KVLORA:KVLORA + 64, :],
                    )
                    nc.gpsimd.dma_start(
                        out=kpe_hi[64:128, cs],
                        in_=agkv_r[r, KVLORA:KVLORA + 64, :],
                    )
                    for m in range(HL):
                        pk = ps_bkv.tile([128, RC], F32, tag="pk")
                        for kc in range(KVFC):
                            nc.tensor.matmul(
                                pk, wkvb_sb[:, kc, m * 128:(m + 1) * 128],
                                kvn_r[:, kc, :],
                                start=(kc == 0), stop=(kc == KVFC - 1),
                            )
                        nc.vector.tensor_copy(out=knT[:, m, cs], in_=pk)
                    for half in range(2):
                        pv = ps_bkv.tile([128, HL * V], F32, tag="pv")
                        for kc in range(KVFC):
                            nc.tensor.matmul(
                                pv, kvn_r[:, kc, half * 128:(half + 1) * 128],
                                wkvb_sb[:, kc, 2 * 128:4 * 128],
                                start=(kc == 0), stop=(kc == KVFC - 1),
                            )
                        nc.vector.tensor_copy(out=v_sb[:, 2 * r + half, :], in_=pv)

                # ---------- q a-pass ----------
                with tc.tile_pool(name="ps_aq", bufs=2, space="PSUM") as ps_aq, \
                     tc.tile_pool(name="ps_aq1", bufs=1, space="PSUM") as ps_aq1:
                    ss_q = ps_aq1.tile([1, RC], F32, tag="ss_q")
                    qraw = sb_a.tile([128, QFC, RC], F32)
                    for g in range(QFC // 2):
                        psA = a_group(ps_aq, wqa_r, g * 2, ss_q, 0, QFC - 1,
                                      "psA_q")
                        for j in range(2):
                            nc.vector.tensor_copy(
                                out=qraw[:, g * 2 + j, :], in_=psA[j]
                            )
                    rstd_q = make_rstd(ss_q, QLORA, "q")
                    for fc in range(QFC):
                        nrm = anorm.tile([128, RC], F32, tag="nrm")
                        nc.vector.scalar_tensor_tensor(
                            out=nrm, in0=qraw[:, fc, :],
                            scalar=lnwq_sb[:, fc:fc + 1], in1=rstd_q,
                            op0=MUL, op1=MUL,
                        )
                        nc.sync.dma_start(out=agq_w[fc], in_=nrm)

                _allgather(nc, tc, D["agq_in"], D["agq_out"])

        # ================= STAGE B: q_b + rope =================
        with tc.tile_pool(name="wqb_p", bufs=1) as wqb_p, \
             tc.tile_pool(name="blk_q", bufs=2) as blk_q, \
             tc.tile_pool(name="qf2_p", bufs=1) as qf2_p, \
             tc.tile_pool(name="ropet", bufs=2) as ropet, \
             tc.tile_pool(name="ps_bq", bufs=2, space="PSUM") as ps_bq, \
             tc.tile_pool(name="ps_rope", bufs=2, space="PSUM") as ps_rope:
            wqb_sb = wqb_p.tile([128, QFC, 3 * 128], F32R)
            nc.sync.dma_start(
                out=wqb_sb,
                in_=P["w_q_b"].ap().rearrange("(kc p) m -> p kc m", p=128),
            )
            qfT2 = qf2_p.tile([128, S], F32R)
            for r in range(NC):
                cs = slice(r * RC, (r + 1) * RC)
                pq = []
                for m in range(3):
                    pq_m = ps_bq.tile([128, RC], F32, tag=f"pq{m}")
                    pq.append(pq_m)
                for half in range(2):
                    qn_h = blk_q.tile([128, QFC // 2, RC], F32R, tag="qn")
                    nc.gpsimd.dma_start(
                        out=qn_h,
                        in_=agq_r[r, half * 6:(half + 1) * 6].transpose([1, 0, 2]),
                    )
                    for m in range(3):
                        for kc in range(QFC // 2):
                            nc.tensor.matmul(
                                pq[m],
                                wqb_sb[:, half * 6 + kc, m * 128:(m + 1) * 128],
                                qn_h[:, kc, :],
                                start=(half == 0 and kc == 0),
                                stop=(half == 1 and kc == QFC // 2 - 1),
                            )
                for m in range(HL):
                    nc.vector.tensor_copy(out=qfT[:, m, cs], in_=pq[m])
                nc.vector.tensor_copy(out=qfT2[:, cs], in_=pq[2])

            for t in range(NQT):
                qs = slice(t * QT, (t + 1) * QT)
                psw = ps_rope.tile([128, QT], F32, tag="qsw")
                nc.tensor.matmul(psw, perm128_sb, qfT2[:, qs],
                                 start=True, stop=True)
                a_c = ropet.tile([128, QT], F32, tag="ropeA")
                nc.sync.dma_start(out=a_c, in_=P["a128"].ap()[:, qs])
                b_c = ropet.tile([128, QT], F32, tag="ropeB")
                nc.sync.dma_start(out=b_c, in_=P["b128"].ap()[:, qs])
                t1q = ropet.tile([128, QT], F32, tag="ropet1")
                nc.vector.tensor_mul(t1q, qfT2[:, qs].bitcast(F32), a_c)
                t2q = ropet.tile([128, QT], F32, tag="ropet2")
                nc.vector.tensor_mul(t2q, psw, b_c)
                nc.vector.tensor_add(qropeP[:, qs], t1q, t2q)

        # ================= attention + o_proj =================
        with tc.tile_pool(name="wo_p", bufs=1) as wo_p, \
             tc.tile_pool(name="p_pool", bufs=2) as p_pool, \
             tc.tile_pool(name="ostr", bufs=2) as ostr, \
             tc.tile_pool(name="ps_att", bufs=2, space="PSUM") as ps_att, \
             tc.tile_pool(name="ps_o", bufs=1, space="PSUM") as ps_o:
            wo_sb = wo_p.tile([128, KCH, OC], F32R)
            nc.sync.dma_start(
                out=wo_sb,
                in_=P["w_o"].ap().rearrange("(kc p) m -> p kc m", p=128),
            )
            for t in range(NQT):
                qs = slice(t * QT, (t + 1) * QT)
                nkc = 4 * (t + 1)
                for h in range(HL):
                    kpe_t = kpe_lo if h == 0 else kpe_hi
                    ps_pv = ps_att.tile([128, QT], F32, tag="ps_pv")
                    ps_sum = ps_att.tile([1, QT], F32, tag="ps_sum")
                    for kc in range(nkc):
                        ks = slice(kc * 128, (kc + 1) * 128)
                        ps_s = ps_att.tile([128, QT], F32, tag="ps_s")
                        nc.tensor.matmul(ps_s, knT[:, h, ks], qfT[:, h, qs],
                                         start=True, stop=False)
                        nc.tensor.matmul(ps_s, kpe_t[:, ks], qropeP[:, qs],
                                         start=False, stop=True)
                        p_sb = p_pool.tile([128, QT], F32R, tag="p_sb")
                        nc.scalar.activation(out=p_sb, in_=ps_s, func=Exp,
                                             scale=SCALING)
                        if kc >= nkc - 4:
                            pm = p_pool.tile([128, QT], F32R, tag="pm")
                            nc.vector.tensor_mul(
                                pm, p_sb, mask_sb[:, kc - (nkc - 4), :]
                            )
                            p_use = pm
                        else:
                            p_use = p_sb
                        nc.tensor.matmul(ps_sum, ones_sb, p_use,
                                         start=(kc == 0), stop=(kc == nkc - 1),
                                         skip_group_check=True)
                        nc.tensor.matmul(ps_pv,
                                         v_sb[:, kc, h * V:(h + 1) * V], p_use,
                                         start=(kc == 0), stop=(kc == nkc - 1),
                                         skip_group_check=True)
                    rinv = rows_p.tile([1, QT], F32, tag="rinv")
                    nc.vector.reciprocal(out=rinv, in_=ps_sum)
                    rb = p_pool.tile([128, QT], F32, tag="rb")
                    nc.gpsimd.partition_broadcast(rb, rinv)
                    atn = p_pool.tile([128, QT], F32, tag="atn")
                    nc.vector.tensor_mul(atn, ps_pv, rb)
                    nc.sync.dma_start(
                        out=D["ag2_in"][t].ap()[h * V:(h + 1) * V, :], in_=atn
                    )
                _allgather(nc, tc, D["ag2_in"][t], D["ag2_out"][t])

                ag2_r = D["ag2_out"][t].ap().rearrange("(hv p) q -> hv p q", p=128)
                for qc in range(QT // 128):
                    at = ostr.tile([128, KCH, 128], F32R, tag="at")
                    nc.gpsimd.dma_start(
                        out=at,
                        in_=ag2_r[:, :, qc * 128:(qc + 1) * 128].transpose(
                            [1, 0, 2]
                        ),
                    )
                    py0 = ps_o.tile([128, OC // 2], F32, tag="py0")
                    py1 = ps_o.tile([128, OC // 2], F32, tag="py1")
                    for hv in range(KCH):
                        nc.tensor.matmul(py0, at[:, hv, :],
                                         wo_sb[:, hv, 0:OC // 2],
                                         start=(hv == 0), stop=(hv == KCH - 1))
                        nc.tensor.matmul(py1, at[:, hv, :],
                                         wo_sb[:, hv, OC // 2:OC],
                                         start=(hv == 0), stop=(hv == KCH - 1))
                    y_sb = ostr.tile([128, OC], F32, tag="y_sb")
                    nc.vector.tensor_copy(out=y_sb[:, 0:OC // 2], in_=py0)
                    nc.vector.tensor_copy(out=y_sb[:, OC // 2:OC], in_=py1)
                    r0 = t * QT + qc * 128
                    nc.sync.dma_start(out=y.ap()[r0:r0 + 128, :], in_=y_sb)


_NC_CACHE = None


def _get_module():
    global _NC_CACHE
    if _NC_CACHE is None:
        _NC_CACHE = _build_module()
    return _NC_CACHE


def _host_prep(positions, hidden_states, w_q_a, q_a_ln_w, w_q_b, w_kv_a,
               kv_a_ln_w, w_kv_b, w_o):
    inv_freq = _yarn_inv_freq()
    pos_f = np.asarray(positions).astype(np.float64)
    freqs = pos_f[:, None] * inv_freq.astype(np.float64)[None, :]
    cosT = (np.cos(freqs) * COS_SIN_MSCALE).astype(np.float32).T.copy()  # [32, S]
    sinT = (np.sin(freqs) * COS_SIN_MSCALE).astype(np.float32).T.copy()

    perm_rope = np.concatenate([np.arange(0, ROPE, 2), np.arange(1, ROPE, 2)])

    w_kv_a_p = np.zeros((HIDDEN, KVLORA + 128), np.float32)
    w_kv_a_p[:, :KVLORA] = w_kv_a[:, :KVLORA]
    w_kv_a_p[:, KVLORA:KVLORA + 64] = w_kv_a[:, KVLORA:][:, perm_rope]
    lnw_q = np.ascontiguousarray(q_a_ln_w.reshape(QFC, 128).T)
    lnw_kv = np.ascontiguousarray(kv_a_ln_w.reshape(KVFC, 128).T)
    a128 = np.ascontiguousarray(np.concatenate([cosT, cosT, cosT, cosT], axis=0))
    b128 = np.ascontiguousarray(np.concatenate([-sinT, sinT, -sinT, sinT], axis=0))
    sw32 = np.zeros((64, 64), np.float32)
    sw32[np.arange(32), np.arange(32) + 32] = 1.0
    sw32[np.arange(32) + 32, np.arange(32)] = 1.0
    perm64 = np.zeros((128, 64), np.float32)
    perm64[0:64, :] = sw32
    perm128 = np.zeros((128, 128), np.float32)
    perm128[0:64, 0:64] = sw32
    perm128[64:128, 64:128] = sw32
    ones_p = np.ones((128, 1), np.float32)
    maskt = np.zeros((128, 4, QT), np.float32)
    kk = np.arange(128)[:, None]
    qq = np.arange(QT)[None, :]
    for d in range(4):
        maskt[:, d, :] = (kk + d * 128 <= qq).astype(np.float32)

    wqg = w_q_b.reshape(QLORA, H, QK)
    wkvg = w_kv_b.reshape(KVLORA, H, NOPE + V)

    in_maps = []
    for c in range(NC):
        h0, h1 = 2 * c, 2 * c + 1
        rows = slice(c * RC, (c + 1) * RC)
        hidden_t = np.ascontiguousarray(hidden_states[rows, :].T)
        rope0 = wqg[:, h0, NOPE:][:, perm_rope]
        rope1 = wqg[:, h1, NOPE:][:, perm_rope]
        w_q_b_c = np.ascontiguousarray(np.concatenate(
            [wqg[:, h0, :NOPE], wqg[:, h1, :NOPE], rope0, rope1], axis=1
        ))
        w_kv_b_c = np.ascontiguousarray(np.concatenate(
            [wkvg[:, h0, :NOPE], wkvg[:, h1, :NOPE],
             wkvg[:, h0, NOPE:], wkvg[:, h1, NOPE:]], axis=1
        ))
        w_o_c = np.ascontiguousarray(w_o[:, c * OC:(c + 1) * OC])
        a64_c = np.ascontiguousarray(
            np.concatenate([cosT[:, rows], cosT[:, rows]], axis=0)
        )
        b64_c = np.ascontiguousarray(
            np.concatenate([-sinT[:, rows], sinT[:, rows]], axis=0)
        )
        in_maps.append({
            "hidden_t": hidden_t, "w_q_a": w_q_a, "w_kv_a": w_kv_a_p,
            "lnw_q": lnw_q, "lnw_kv": lnw_kv, "w_q_b": w_q_b_c,
            "w_kv_b": w_kv_b_c, "w_o": w_o_c, "a64": a64_c, "b64": b64_c,
            "a128": a128, "b128": b128, "perm64": perm64, "perm128": perm128,
            "ones_p": ones_p, "maskt": maskt,
        })
    return in_maps


def kernel(positions, hidden_states, w_q_a, q_a_ln_w, w_q_b, w_kv_a, kv_a_ln_w,
           w_kv_b, w_o):
    args = [np.asarray(x, dtype=np.float32) for x in
            (hidden_states, w_q_a, q_a_ln_w, w_q_b, w_kv_a, kv_a_ln_w, w_kv_b,
             w_o)]
    in_maps = _host_prep(np.asarray(positions), *args)
    nc = _get_module()
    res = run_bass_kernel_spmd(nc, in_maps, list(range(NC)))
    out = np.concatenate([res.results[c]["y"] for c in range(NC)], axis=1)
    return np.ascontiguousarray(out.astype(np.float32))


if __name__ == "__main__":
    rng = np.random.default_rng(0)
    inputs = {
        "positions": np.arange(S, dtype=np.int32),
        "hidden_states": rng.standard_normal((S, HIDDEN), dtype=np.float32),
        "w_q_a": rng.standard_normal((HIDDEN, QLORA), dtype=np.float32) * 0.02,
        "q_a_ln_w": np.ones(QLORA, np.float32),
        "w_q_b": rng.standard_normal((QLORA, H * QK), dtype=np.float32) * 0.02,
        "q_kv": None,
    }
    del inputs["q_kv"]
    inputs["w_kv_a"] = rng.standard_normal((HIDDEN, KVLORA + ROPE),
                                           dtype=np.float32) * 0.02
    inputs["kv_a_ln_w"] = np.ones(KVLORA, np.float32)
    inputs["w_kv_b"] = rng.standard_normal((KVLORA, H * (NOPE + V)),
                                           dtype=np.float32) * 0.02
    inputs["w_o"] = rng.standard_normal((H * V, HIDDEN), dtype=np.float32) * 0.02
    out = kernel(**inputs)
    print("kernel out", out.shape, out.dtype, float(np.abs(out).mean()))
